# revision 5
# baseline (speedup 1.0000x reference)
"""Causal self-attention (B=2, T=2048, D=2048, H=16) on 8 TRN2 NeuronCores.

Sharding: 2D — batch x head-groups. Core c handles batch c//4 and heads
[4*(c%4) .. 4*(c%4)+3]. Each core computes the partial output (its 4 heads'
contribution through the output projection) for its batch in bf16; the host
sums the 4 partials per batch in fp32.

Per-core kernel:
  P1  QKV projection: x and weights stream in bf16 (halves HBM traffic and
      enables fast weight loads); Q^T/K^T produced in [hd, t] rope-split
      layout via DVE rope (4 ops per tile, sign of sin folded into the
      table), V natural [t, f]. DMA FIFO ordered so the first x tiles land
      before the bulk weights.
  P2  Flash-style causal attention per 512-wide q-window: S^T = K_blk Q^T
      (PSUM), E^T = exp(scale*S^T) as bf16 via ACT, diagonal masking on DVE,
      O^T += V.T E^T accumulated in [hd, q] layout (N=512 matmuls, no
      transposes), softmax denominator via an all-ones stationary matmul,
      normalization = DVE reciprocal + multiply.
  P3  Output projection interleaved per q-window so it overlaps later
      attention windows; output staged bf16 and written as 512KB rows.

All matmuls bf16 (1 cyc/row) except S^T which runs f32r for logit accuracy.
"""
import numpy as np
import ml_dtypes

import concourse.bass as bass
from concourse import bacc
import concourse.tile as tile
from concourse import mybir
from concourse.bass_utils import run_bass_kernel_spmd

B, T, D, H = 2, 2048, 2048, 16
HD = D // H            # 128
HPC = 4                # heads per core
FL = HPC * HD          # local features = 512
QW = 512               # q-window for attention
NQW = T // QW          # 4
SCALE = float(1.0 / np.sqrt(np.float32(HD)))

f32 = mybir.dt.float32
f32r = mybir.dt.float32r
bf16 = mybir.dt.bfloat16

_BUILT = {}


def _build_nc():
    nc = bacc.Bacc()

    xT = nc.dram_tensor("xT", (D, T), bf16, kind="ExternalInput")
    wqk = nc.dram_tensor("wqk", (D, 2 * FL), bf16, kind="ExternalInput")
    wv = nc.dram_tensor("wv", (D, FL), bf16, kind="ExternalInput")
    wp = nc.dram_tensor("wp", (FL, D), bf16, kind="ExternalInput")
    cos2 = nc.dram_tensor("cos2", (HD, T), f32, kind="ExternalInput")
    sin2 = nc.dram_tensor("sin2", (HD, T), f32, kind="ExternalInput")
    dmask = nc.dram_tensor("dmask", (4, 128, QW), bf16, kind="ExternalInput")
    onesmb = nc.dram_tensor("onesmb", (128, 128), bf16, kind="ExternalInput")
    outp = nc.dram_tensor("outp", (T, D), bf16, kind="ExternalOutput")
    tick = nc.dram_tensor("tick", (128, 8), f32, kind="ExternalInput")
    tock = nc.dram_tensor("tock", (128, 8), f32, kind="ExternalOutput")

    with tile.TileContext(nc) as tc:
        from contextlib import ExitStack
        with ExitStack() as top:
            pers = top.enter_context(tc.tile_pool(name="pers", bufs=1))
            # persistent weights / rope / masks / ones
            wqk_sb = [pers.tile([128, 2 * FL], bf16, tag=f"wqk{k}", name=f"wqk{k}")
                      for k in range(16)]
            wv_sb = [pers.tile([128, FL], bf16, tag=f"wv{k}", name=f"wv{k}")
                     for k in range(16)]
            tick_sb = pers.tile([128, 8], f32, tag="tick")
            nc.sync.dma_start(out=tick_sb, in_=tick[:, :])
            nc.sync.dma_start(out=tock[:, :], in_=tick_sb)

            # per-core persistent: Q^T/K^T (f32r), V tiles (bf16), attn^T (bf16)
            qkT = [pers.tile([HD, T], f32r, tag=f"qkT{f}", name=f"qkT{f}")
                   for f in range(2 * HPC)]          # 0..3 = q heads, 4..7 = k heads
            vP = [pers.tile([128, FL], bf16, tag=f"vP{i}", name=f"vP{i}")
                  for i in range(T // 128)]
            aT = [pers.tile([128, T], bf16, tag=f"aT{h}", name=f"aT{h}")
                  for h in range(HPC)]

            # wp loaded early (pers) so P3 can interleave with P2
            wp_sb = [pers.tile([128, D], bf16, tag=f"wp{f}", name=f"wp{f}")
                     for f in range(HPC)]

            # ---- P1: QKV projection + rope ----
            with ExitStack() as p1:
                rope_p = p1.enter_context(tc.tile_pool(name="ropep", bufs=1))
                xp = p1.enter_context(tc.tile_pool(name="xp", bufs=23))
                rp = p1.enter_context(tc.tile_pool(name="rp", bufs=2))
                ps_qk = p1.enter_context(
                    tc.tile_pool(name="psqk", bufs=6, space="PSUM"))
                ps_v = p1.enter_context(
                    tc.tile_pool(name="psv", bufs=2, space="PSUM"))
                cos_sb = rope_p.tile([HD, T], f32, tag="cos")
                sin_sb = rope_p.tile([HD, T], f32, tag="sin")
                # DMA FIFO order matters: interleave quarter-0 x tiles with
                # wqk so the first QK accumulation starts ~30us earlier; bulk
                # weights (wv, wp) follow behind.
                xk0 = []
                for k in range(16):
                    xt = xp.tile([128, QW], bf16, tag="xt", name="xt")
                    nc.sync.dma_start(out=xt, in_=xT[k * 128:(k + 1) * 128, 0:QW])
                    nc.sync.dma_start(
                        out=wqk_sb[k], in_=wqk[k * 128:(k + 1) * 128, :])
                    xk0.append(xt)
                nc.sync.dma_start(out=cos_sb, in_=cos2[:, :])
                nc.sync.dma_start(out=sin_sb, in_=sin2[:, :])
                for k in range(16):
                    nc.sync.dma_start(out=wv_sb[k], in_=wv[k * 128:(k + 1) * 128, :])
                for f in range(HPC):
                    nc.sync.dma_start(out=wp_sb[f], in_=wp[f * 128:(f + 1) * 128, :])
                for th in range(4):            # t-quarters of 512
                    t0 = th * QW
                    if th == 0:
                        xk = xk0
                    else:
                        xk = []
                        for k in range(16):
                            xt = xp.tile([128, QW], bf16, tag="xt", name="xt")
                            nc.sync.dma_start(
                                out=xt, in_=xT[k * 128:(k + 1) * 128, t0:t0 + QW])
                            xk.append(xt)
                    # Q^T / K^T f-blocks with rope (sin2 has -sin in rows 0:64)
                    for fb in range(2 * HPC):
                        ps = ps_qk.tile([128, QW], f32, tag="psqk")
                        for k in range(16):
                            nc.tensor.matmul(
                                ps[:, :],
                                wqk_sb[k][:, fb * 128:(fb + 1) * 128],
                                xk[k][:, :],
                                start=(k == 0), stop=(k == 15))
                        dsl = qkT[fb][:, t0:t0 + QW]
                        ca = cos_sb[:, t0:t0 + QW]
                        sa = sin_sb[:, t0:t0 + QW]
                        ta = rp.tile([128, QW], f32, tag="ra")
                        tb = rp.tile([128, QW], f32, tag="rb")
                        nc.vector.tensor_mul(ta, ps[:, :], ca)
                        nc.vector.tensor_mul(
                            tb[0:64, :], ps[64:128, :], sa[0:64, :])
                        nc.vector.tensor_mul(
                            tb[64:128, :], ps[0:64, :], sa[64:128, :])
                        nc.vector.tensor_add(dsl, ta, tb)
                    # V natural [t, f]
                    for tb4 in range(4):
                        ps = ps_v.tile([128, FL], f32, tag="psv")
                        for k in range(16):
                            nc.tensor.matmul(
                                ps[:, :],
                                xk[k][:, tb4 * 128:(tb4 + 1) * 128],
                                wv_sb[k][:, :],
                                start=(k == 0), stop=(k == 15))
                        nc.vector.tensor_copy(vP[(t0 // 128) + tb4], ps[:, :])

            # ---- P2 + P3 interleaved per q-window ----
            with ExitStack() as p2:
                mp = p2.enter_context(tc.tile_pool(name="mp", bufs=1))
                ep = p2.enter_context(tc.tile_pool(name="ep", bufs=4))
                sp = p2.enter_context(tc.tile_pool(name="sp", bufs=4))
                op = p2.enter_context(tc.tile_pool(name="op", bufs=3))
                ps_s = p2.enter_context(
                    tc.tile_pool(name="pss", bufs=3, space="PSUM"))
                ps_o = p2.enter_context(
                    tc.tile_pool(name="pso", bufs=2, space="PSUM"))
                ps_d = p2.enter_context(
                    tc.tile_pool(name="psd", bufs=1, space="PSUM"))
                ps_p = p2.enter_context(
                    tc.tile_pool(name="psp", bufs=2, space="PSUM"))
                msk_sb = [mp.tile([128, QW], bf16, tag=f"msk{d}", name=f"msk{d}")
                          for d in range(4)]
                onesm = mp.tile([128, 128], bf16, tag="onesm")
                for d in range(4):
                    nc.sync.dma_start(out=msk_sb[d], in_=dmask[d, :, :])
                nc.sync.dma_start(out=onesm, in_=onesmb[:, :])
                for w in range(NQW):
                    q0 = w * QW
                    nsb = (w + 1) * 4
                    for h in range(HPC):
                        po = ps_o.tile([128, QW], f32, tag="pso")
                        dsum = ps_d.tile([128, QW], f32, tag="psd")
                        for sb in range(nsb):
                            s0 = sb * 128
                            # diagonal blocks: only q >= s is live. Compute
                            # S/exp/mask from qs (kept >= 256 wide so the
                            # f32r moving stream stays at 1 cyc/row); feed
                            # PV/dsum from qv (mask zeroes [qs, qv)).
                            d = (s0 - q0) // 128 if s0 >= q0 else -1
                            qv = 0 if d < 0 else d * 128
                            qs = 0 if d < 0 else min(d * 128, QW - 256)
                            ps = ps_s.tile([128, QW], f32, tag="pss")
                            nc.tensor.matmul(
                                ps[:, qs:QW],
                                qkT[HPC + h][:, s0:s0 + 128],
                                qkT[h][:, q0 + qs:q0 + QW],
                                start=True, stop=True)
                            et = ep.tile([128, QW], bf16, tag="et")
                            nc.scalar.activation(
                                et[:, qs:QW], ps[:, qs:QW],
                                mybir.ActivationFunctionType.Exp, scale=SCALE)
                            if d >= 0:
                                nc.vector.tensor_mul(
                                    et[:, qs:QW], et[:, qs:QW],
                                    msk_sb[d][:, qs:QW])
                            nc.tensor.matmul(
                                po[:, qv:QW],
                                vP[sb][:, h * 128:(h + 1) * 128],
                                et[:, qv:QW],
                                start=(sb == 0), stop=(sb == nsb - 1))
                            nc.tensor.matmul(
                                dsum[:, qv:QW],
                                onesm[:, :],
                                et[:, qv:QW],
                                start=(sb == 0), stop=(sb == nsb - 1))
                        # normalize: every dsum row holds the column sums
                        rin = sp.tile([128, QW], f32, tag="rin")
                        nc.vector.reciprocal(rin, dsum[:, :])
                        nc.vector.tensor_mul(
                            aT[h][:, q0:q0 + QW], po[:, :], rin[:, :])
                    # P3 for this window's t-blocks (all heads' aT ready)
                    for tb in range(w * 4, (w + 1) * 4):
                        otb = op.tile([128, D], bf16, tag="otb")
                        for ec in range(4):
                            ps = ps_p.tile([128, 512], f32, tag="psp")
                            for fk in range(HPC):
                                nc.tensor.matmul(
                                    ps[:, :],
                                    aT[fk][:, tb * 128:(tb + 1) * 128],
                                    wp_sb[fk][:, ec * 512:(ec + 1) * 512],
                                    start=(fk == 0), stop=(fk == HPC - 1))
                            nc.scalar.activation(
                                otb[:, ec * 512:(ec + 1) * 512], ps[:, :],
                                mybir.ActivationFunctionType.Copy)
                        nc.sync.dma_start(
                            out=outp[tb * 128:(tb + 1) * 128, :], in_=otb)
    nc.finalize()
    return nc


def _prep_in_maps(x, rope, mask, w_attn, w_proj):
    x = np.asarray(x, dtype=np.float32)
    rope = np.asarray(rope, dtype=np.float32)
    mask = np.asarray(mask)
    w_attn = np.asarray(w_attn, dtype=np.float32)
    w_proj = np.asarray(w_proj, dtype=np.float32)

    xTb = [np.ascontiguousarray(x[b].T).astype(ml_dtypes.bfloat16) for b in range(B)]
    cosT = np.ascontiguousarray(rope[:, :, 0].T)  # (64, T)
    sinT = np.ascontiguousarray(rope[:, :, 1].T)
    cos2 = np.concatenate([cosT, cosT], axis=0).astype(np.float32)
    # sign folded: rows 0:64 hold -sin so rope is one full-width add
    sin2 = np.concatenate([-sinT, sinT], axis=0).astype(np.float32)

    # diagonal mask tiles in S^T layout from the provided mask
    m512 = np.asarray(mask[0, 0, :QW, :QW])
    dm = np.zeros((4, 128, QW), dtype=np.float32)
    for d in range(4):
        for i in range(128):
            dm[d, i, :] = m512[:, i + d * 128].astype(np.float32)
    dm = dm.astype(ml_dtypes.bfloat16)

    perm = np.concatenate([np.arange(0, HD, 2), np.arange(1, HD, 2)])
    onesmb = np.ones((128, 128), dtype=ml_dtypes.bfloat16)
    in_maps = []
    for c in range(8):
        b = c // 4
        hg = c % 4
        heads = [4 * hg + i for i in range(HPC)]
        qrows = np.concatenate([w_attn[h * HD:(h + 1) * HD][perm] for h in heads])
        krows = np.concatenate(
            [w_attn[D + h * HD:D + (h + 1) * HD][perm] for h in heads])
        vrows = np.concatenate(
            [w_attn[2 * D + h * HD:2 * D + (h + 1) * HD] for h in heads])
        wqk_c = np.ascontiguousarray(
            np.concatenate([qrows, krows]).T).astype(ml_dtypes.bfloat16)  # (D, 1024)
        wv_c = np.ascontiguousarray(vrows.T).astype(ml_dtypes.bfloat16)   # (D, 512)
        wp_c = np.ascontiguousarray(
            w_proj[:, hg * FL:(hg + 1) * FL].T).astype(ml_dtypes.bfloat16)  # (512, D)
        im = {"xT": xTb[b], "wqk": wqk_c, "wv": wv_c, "wp": wp_c,
              "cos2": cos2, "sin2": sin2, "dmask": dm,
              "onesmb": onesmb,
              "tick": np.zeros((128, 8), np.float32)}
        in_maps.append(im)
    return in_maps


def kernel(x, rope, mask, w_attn, w_proj):
    if "nc" not in _BUILT:
        _BUILT["nc"] = _build_nc()
    nc = _BUILT["nc"]
    in_maps = _prep_in_maps(x, rope, mask, w_attn, w_proj)
    res = run_bass_kernel_spmd(nc, in_maps, core_ids=list(range(8)))
    out = np.zeros((B, T, D), dtype=np.float64)
    for c in range(8):
        out[c // 4] += res.results[c]["outp"].astype(np.float64)
    return out.astype(np.float32)
